# revision 1
# baseline (speedup 1.0000x reference)
"""CompressedKVAttention Trainium2 Bass kernel.

GQA attention with int8-quantized KV caches, per-(b, kv_head, token)
scale/zero dequant params.  B=4, H=32, HKV=8, QLEN=16, KVLEN=8192, D=128.

Sharding: B x HKV = 32 (batch, kv-head) groups, 4 per core across 8 cores
(data parallel on B, tensor parallel on kv-head groups).  No cross-device
comms.

Per-group math (q' = n_rep*qlen = 64 query rows, t = kv position):
  khat[d, t]  = scale_k[t] * (K[t, d] - zero_k[t]) / sqrt(D)     (host, fp16)
  S[t, q']    = sum_d khat[d, t] * qT[d, q']                     (PE, PSUM fp32)
  w[t, q']    = exp(S[t, q'])                                    (ACT, fp16)
  vd[t, d]    = (V[t, d] - zero_v[t]) * scale_v[t]               (DVE, fp16)
  num[q', d]  = sum_t w[t, q'] * vd[t, d]                        (PE, accum)
  den[q']     = sum_t w[t, q']                                   (PE, ones col)
  out[q', d]  = num[q', d] / den[q']                             (DVE)

Scores are computed transposed ([t, q']) so the per-token dequant params are
per-partition scalars.  K is pre-transposed/dequantized on the host (bf16
would double int8 K traffic regardless of where the transpose happens, and
the PE needs d on partitions); V stays int8 end-to-end and is dequantized
on-chip with a single fused tensor_scalar per tile.
"""

import numpy as np

B, H, HKV, QLEN, KVLEN, D = 4, 32, 8, 16, 8192, 128
NREP = H // HKV            # 4 query heads per kv head
QP = NREP * QLEN           # 64 query rows per group
NCORES = 8
GPC = (B * HKV) // NCORES  # 4 groups per core
TT = 128                   # kv tokens per tile
SLAB = 8                   # score tiles per psum bank slab

_cached = {}


def _build_nc(kvlen, gpc, debug=False):
    import concourse.bacc as bacc
    import concourse.tile as tile
    from concourse import mybir

    nt = kvlen // TT           # t-tiles per group
    nslab = max(1, nt // SLAB)
    slab = min(SLAB, nt)
    kh_ch = min(2048, kvlen)   # khat dma chunk width (t)
    vq_ch = min(4096, kvlen)   # v dma chunk width

    nc = bacc.Bacc("TRN2", target_bir_lowering=False, debug=debug)
    f16, f32, i8 = mybir.dt.float16, mybir.dt.float32, mybir.dt.int8

    khat_d = nc.dram_tensor("khat", [gpc, D, kvlen], f16, kind="ExternalInput")
    qhat_d = nc.dram_tensor("qhat", [gpc, D, QP], f16, kind="ExternalInput")
    vq_d = nc.dram_tensor("vq", [gpc, TT, (kvlen // TT) * D], i8, kind="ExternalInput")
    svp_d = nc.dram_tensor("svp", [gpc, TT, nt], f32, kind="ExternalInput")
    zvp_d = nc.dram_tensor("zvp", [gpc, TT, nt], f32, kind="ExternalInput")
    out_d = nc.dram_tensor("out", [gpc, QP, D], f32, kind="ExternalOutput")

    with tile.TileContext(nc) as tc:
        with (
            tc.tile_pool(name="kh", bufs=8) as kh_pool,
            tc.tile_pool(name="vqp", bufs=4) as vq_pool,
            tc.tile_pool(name="vd", bufs=16) as vd_pool,
            tc.tile_pool(name="w", bufs=4) as w_pool,
            tc.tile_pool(name="vec", bufs=3) as vec_pool,
            tc.tile_pool(name="const", bufs=1) as const_pool,
            tc.tile_pool(name="obuf", bufs=4) as o_pool,
            tc.tile_pool(name="ps", bufs=4, space="PSUM") as ps_pool,
            tc.tile_pool(name="pnum", bufs=2, space="PSUM") as pnum_pool,
            tc.tile_pool(name="pden", bufs=2, space="PSUM") as pden_pool,
        ):
            ones_t = const_pool.tile([TT, 1], f16)
            nc.vector.memset(ones_t[:], 1.0)

            for g in range(gpc):
                # group input loads
                kh_chunks = []
                for c in range(kvlen // kh_ch):
                    t = kh_pool.tile([D, kh_ch], f16, tag="kh")
                    nc.sync.dma_start(
                        out=t[:], in_=khat_d[g, :, c * kh_ch : (c + 1) * kh_ch]
                    )
                    kh_chunks.append(t)
                vq_chunks = []
                for c in range(kvlen // vq_ch):
                    t = vq_pool.tile([TT, (vq_ch // TT) * D], i8, tag="vqp")
                    nc.sync.dma_start(
                        out=t[:],
                        in_=vq_d[g, :, c * (vq_ch // TT) * D : (c + 1) * (vq_ch // TT) * D],
                    )
                    vq_chunks.append(t)
                sv_t = vec_pool.tile([TT, nt], f32, tag="sv")
                nc.sync.dma_start(out=sv_t[:], in_=svp_d[g])
                zv_t = vec_pool.tile([TT, nt], f32, tag="zv")
                nc.sync.dma_start(out=zv_t[:], in_=zvp_d[g])
                qh_t = vec_pool.tile([D, QP], f16, tag="qh")
                nc.sync.dma_start(out=qh_t[:], in_=qhat_d[g])

                psum_num = pnum_pool.tile([QP, D], mybir.dt.float32)
                psum_den = pden_pool.tile([QP, 1], mybir.dt.float32)

                for s in range(nslab):
                    ps = ps_pool.tile([TT, slab * QP], mybir.dt.float32)
                    for j in range(slab):
                        i = s * slab + j
                        ck, off = divmod(i * TT, kh_ch)
                        nc.tensor.matmul(
                            ps[:, j * QP : (j + 1) * QP],
                            lhsT=kh_chunks[ck][:, off : off + TT],
                            rhs=qh_t[:],
                            start=True,
                            stop=True,
                        )
                    w_t = w_pool.tile([TT, slab * QP], f16, tag="w")
                    nc.scalar.activation(
                        out=w_t[:], in_=ps[:], func=mybir.ActivationFunctionType.Exp
                    )
                    for j in range(slab):
                        i = s * slab + j
                        vd_t = vd_pool.tile([TT, D], f16, tag="vd")
                        ck, off = divmod(i * D, (vq_ch // TT) * D)
                        nc.vector.tensor_scalar(
                            out=vd_t[:],
                            in0=vq_chunks[ck][:, off : off + D],
                            scalar1=zv_t[:, i : i + 1],
                            scalar2=sv_t[:, i : i + 1],
                            op0=mybir.AluOpType.subtract,
                            op1=mybir.AluOpType.mult,
                        )
                        nc.tensor.matmul(
                            psum_num[:],
                            lhsT=w_t[:, j * QP : (j + 1) * QP],
                            rhs=vd_t[:],
                            start=(i == 0),
                            stop=(i == nt - 1),
                        )
                        nc.tensor.matmul(
                            psum_den[:],
                            lhsT=w_t[:, j * QP : (j + 1) * QP],
                            rhs=ones_t[:],
                            start=(i == 0),
                            stop=(i == nt - 1),
                        )

                rec_t = o_pool.tile([QP, 1], f32, tag="rec")
                nc.vector.reciprocal(out=rec_t[:], in_=psum_den[:])
                o_t = o_pool.tile([QP, D], f32, tag="o")
                nc.vector.tensor_scalar(
                    out=o_t[:],
                    in0=psum_num[:],
                    scalar1=rec_t[:],
                    scalar2=None,
                    op0=mybir.AluOpType.mult,
                )
                nc.sync.dma_start(out=out_d[g], in_=o_t[:])

    nc.compile()
    return nc


def _host_prep(query, key_cache, value_cache, key_scale, key_zero,
               value_scale, value_zero, kvlen=KVLEN, ncores=NCORES, gpc=GPC):
    """Build per-core input maps. Groups are (b, kv_head) pairs, flat index
    b*HKV + kvh, gpc consecutive groups per core."""
    nt = kvlen // TT
    scale = 1.0 / np.sqrt(D)
    in_maps = []
    for c in range(ncores):
        khat = np.empty((gpc, D, kvlen), np.float16)
        qhat = np.empty((gpc, D, QP), np.float16)
        vqp = np.empty((gpc, TT, nt * D), np.int8)
        svp = np.empty((gpc, TT, nt), np.float32)
        zvp = np.empty((gpc, TT, nt), np.float32)
        for g in range(gpc):
            flat = c * gpc + g
            b, kvh = divmod(flat, HKV)
            k = key_cache[b, kvh].astype(np.float32)          # [t, d]
            kz = key_zero[b, kvh][:, None]
            ks = key_scale[b, kvh][:, None]
            khat[g] = ((k - kz) * (ks * scale)).T.astype(np.float16)
            q = query[b, kvh * NREP : (kvh + 1) * NREP]        # [nrep, qlen, d]
            qhat[g] = q.reshape(QP, D).T.astype(np.float16)
            # v p-major: vqp[p, i*D + d] = V[i*TT + p, d]
            vqp[g] = (
                value_cache[b, kvh]
                .reshape(nt, TT, D)
                .transpose(1, 0, 2)
                .reshape(TT, nt * D)
            )
            svp[g] = value_scale[b, kvh].reshape(nt, TT).T
            zvp[g] = value_zero[b, kvh].reshape(nt, TT).T
        in_maps.append(
            {"khat": khat, "qhat": qhat, "vq": vqp, "svp": svp, "zvp": zvp}
        )
    return in_maps


def _host_post(results, ncores=NCORES, gpc=GPC):
    out = np.empty((B, H, QLEN, D), np.float32)
    for c in range(ncores):
        o = results[c]["out"]  # [gpc, QP, D]
        for g in range(gpc):
            flat = c * gpc + g
            b, kvh = divmod(flat, HKV)
            out[b, kvh * NREP : (kvh + 1) * NREP] = o[g].reshape(NREP, QLEN, D)
    return out


def kernel(query, key_cache, value_cache, key_scale, key_zero,
           value_scale, value_zero):
    from concourse.bass_utils import run_bass_kernel_spmd

    if "nc" not in _cached:
        _cached["nc"] = _build_nc(KVLEN, GPC)
    nc = _cached["nc"]
    in_maps = _host_prep(
        np.asarray(query), np.asarray(key_cache), np.asarray(value_cache),
        np.asarray(key_scale), np.asarray(key_zero),
        np.asarray(value_scale), np.asarray(value_zero),
    )
    res = run_bass_kernel_spmd(nc, in_maps, core_ids=list(range(NCORES)))
    return _host_post(res.results)
